# revision 8
# baseline (speedup 1.0000x reference)
"""Trainium2 Bass kernel for nn_Connection_75411035783724 (Mamba2 block + MLP head).

Sharding: tensor-parallel over the 32 Mamba2 heads across 8 cores (4 heads each).
Launch 1 (per core): in_proj column slice (x 256 + B 128 + dt 4), causal conv as
accumulating diagonal matmuls, chunked-SSD scan (chunk 256 == position stride).
Pipeline is software-pipelined so the PE never idles: dt matmuls lead each
1024-token batch, the scan matmuls for a batch-pair are emitted one pair later,
and the inter-chunk recurrence runs incrementally on the vector engine
(scalar_tensor_tensor per head/chunk reading scan PSUM directly).  All
position-wise tail quantities (C32/B32/x32/z/dt32/dAA/f4) are computed up front
from 4-token window matmuls. Launch 2: gated RMSNorm + out_proj + MLP on the 32
frame-start rows (column-sharded MLP2).
"""
import os
import sys
import numpy as np
import ml_dtypes

sys.path.insert(0, "/opt/trn_rl_repo")

import concourse.bass as bass
import concourse.tile as tile
import concourse.mybir as mybir
from concourse import bacc
from concourse import bass_utils

F32 = mybir.dt.float32
BF16 = mybir.dt.bfloat16
AF = mybir.ActivationFunctionType
OP = mybir.AluOpType
BF = ml_dtypes.bfloat16

# Model dims
D_MODEL = 1024
HIDDEN = 4096
D_STATE = 128       # n
D_CONV = 4
D_INNER = 2048
HEADDIM = 64        # p
NHEADS = 32
CONV_DIM = D_INNER + 2 * D_STATE            # 2304
D_IN_PROJ = 2 * D_INNER + 2 * D_STATE + NHEADS  # 4384
L = 8192            # tokens
NPOS = 32           # output positions (first token of each frame)
POS_STRIDE = 256
NCORES = 8
HPC = 4             # heads per core
Q = 256             # chunk length
NCHUNK = L // Q     # 32
KT = D_MODEL // 128  # 8 K-tiles
GSZ = 512           # group (one in_proj psum tile span)
BSZ = 1024          # batch = 2 groups (dt pipe granularity)
NB = L // BSZ       # 8 batches
CPB = BSZ // Q      # 4 chunks per batch
PAIR = 2048         # pair = 2 batches (scan-matmul granularity)
NPAIR = L // PAIR   # 4
NCOL = 256 + 128    # in_proj col slice per core: [x 256 | B 128]
MT_SPEC = [(0, 128), (128, 128), (256, 128)]  # (col0, width)
HPN = HEADDIM * NCHUNK  # 2048 (per-head span in Sg free dim)


def _bf(x):
    return np.ascontiguousarray(np.asarray(x, dtype=np.float32)).astype(BF)


def _f32(x):
    return np.ascontiguousarray(np.asarray(x, dtype=np.float32))


# ----------------------------------------------------------------------------
# Launch 1 program: in_proj + conv + scan -> gated y at the 32 positions
# ----------------------------------------------------------------------------
_L1 = None


def build_l1():
    global _L1
    if _L1 is not None:
        return _L1
    nc = bacc.Bacc("TRN2", target_bir_lowering=False, debug=False,
                   num_devices=NCORES)

    def din(name, shape, dt):
        return nc.dram_tensor(name, shape, dt, kind="ExternalInput").ap()

    xT = din("xT", (D_MODEL, L), BF16)
    xTpos = din("xTpos", (D_MODEL, NPOS), BF16)
    xTwin = din("xTwin", (D_MODEL, NPOS * D_CONV), BF16)
    w_in = din("w_in", (KT, 128, NCOL), BF16)
    w_dt = din("w_dt", (KT, 128, HPC), BF16)
    w_c = din("w_c", (KT, 128, 128), BF16)
    w_z = din("w_z", (KT, 128, 256), BF16)
    diag_w = din("diag_w", (3, D_CONV, 128, 128), BF16)
    cw_c = din("cw_c", (128, D_CONV), F32)
    cw3 = din("cw3", (128, 3, D_CONV), F32)
    conv_b = din("conv_b", (128, 3), F32)
    conv_b_c = din("conv_b_c", (128, 1), F32)
    dtb4 = din("dtb4", (HPC, 1), F32)
    A4 = din("A4", (HPC, 1), F32)
    D4 = din("D4", (HPC, 1), F32)
    y32g_out = nc.dram_tensor("y32g", (128, 2, NPOS), F32,
                              kind="ExternalOutput").ap()

    with tile.TileContext(nc) as tc:
        import contextlib
        with contextlib.ExitStack() as ctx:
            sb = ctx.enter_context(tc.tile_pool(name="sb", bufs=1))
            ring = ctx.enter_context(tc.tile_pool(name="ring", bufs=1))
            dsc = ctx.enter_context(tc.tile_pool(name="dsc", bufs=1, space="DRAM"))
            psA = ctx.enter_context(tc.tile_pool(name="psA", bufs=1, space="PSUM"))

            # ---- resident weights/constants (sync queue: loop-critical order)
            w_dt_sb = sb.tile([128, KT, HPC], BF16)
            nc.sync.dma_start(out=w_dt_sb, in_=w_dt.rearrange("k p c -> p k c"))
            xt_slots = [sb.tile([128, KT, BSZ], BF16, name=f"xt{i}")
                        for i in range(2)]
            nc.sync.dma_start(
                out=xt_slots[0],
                in_=xT.rearrange("(k p) t -> p k t", p=128)[:, :, 0:BSZ])
            w_in_sb = sb.tile([128, KT, NCOL], BF16)
            nc.sync.dma_start(out=w_in_sb, in_=w_in.rearrange("k p c -> p k c"))
            dtb_sb = sb.tile([HPC, 1], F32)
            nc.sync.dma_start(out=dtb_sb, in_=dtb4)
            A_sb = sb.tile([HPC, 1], F32)
            nc.sync.dma_start(out=A_sb, in_=A4)
            diag_sb = sb.tile([128, 3, D_CONV, 128], BF16)
            nc.sync.dma_start(out=diag_sb, in_=diag_w.rearrange("c j a b -> a c j b"))
            cb_sb = sb.tile([128, 3], F32)
            nc.sync.dma_start(out=cb_sb, in_=conv_b)
            D_sb = sb.tile([HPC, 1], F32)
            nc.sync.dma_start(out=D_sb, in_=D4)
            # position-block inputs on the gpsimd queue (off the xt stream)
            xtp_sb = sb.tile([128, KT, NPOS], BF16)
            nc.gpsimd.dma_start(out=xtp_sb,
                                in_=xTpos.rearrange("(k p) t -> p k t", p=128))
            xtw_sb = sb.tile([128, KT, NPOS * D_CONV], BF16)
            nc.gpsimd.dma_start(out=xtw_sb,
                                in_=xTwin.rearrange("(k p) t -> p k t", p=128))
            w_c_sb = sb.tile([128, KT, 128], BF16)
            nc.gpsimd.dma_start(out=w_c_sb, in_=w_c.rearrange("k p c -> p k c"))
            w_z_sb = sb.tile([128, KT, 256], BF16)
            nc.gpsimd.dma_start(out=w_z_sb, in_=w_z.rearrange("k p c -> p k c"))
            cwc_sb = sb.tile([128, D_CONV], F32)
            nc.gpsimd.dma_start(out=cwc_sb, in_=cw_c)
            cw3_sb = sb.tile([128, 3, D_CONV], F32)
            nc.gpsimd.dma_start(out=cw3_sb, in_=cw3)
            cbc_sb = sb.tile([128, 1], F32)
            nc.gpsimd.dma_start(out=cbc_sb, in_=conv_b_c)

            # ---- persistent buffers
            xbcc = [ring.tile([128, 3, PAIR], BF16, name=f"xbcc{i}")
                    for i in range(2)]                     # conv+silu [x0|x1|B]
            XT_r = [ring.tile([128, 16, 256], BF16, name=f"XTr{i}")
                    for i in range(2)]                     # transposed w*x (pair)
            BT_r = [ring.tile([128, 16, 128], BF16, name=f"BTr{i}")
                    for i in range(2)]                     # transposed B (pair)
            Sg = sb.tile([128, HPC * HPN], F32)            # running states (h,p,c)
            lamA = sb.tile([HPC, NCHUNK], F32)             # per-chunk decay (log->lin)
            lam_bc = sb.tile([128, HPC, NCHUNK], F32)      # lamA bcast over parts
            # dt pipe work tiles (partition 0-3)
            ones4 = sb.tile([HPC, BSZ], F32)
            dt2_slots = [sb.tile([HPC, BSZ], F32, name=f"dt2{i}") for i in range(2)]
            a2_s = sb.tile([HPC, BSZ], F32)
            s2_s = sb.tile([HPC, BSZ], F32)
            w2b_slots = [sb.tile([HPC, BSZ], BF16, name=f"w2b{i}") for i in range(2)]
            nc.vector.memset(ones4, 1.0)
            z_ap = bass.AP(tensor=ones4.tensor, offset=ones4.offset,
                           ap=[list(ones4.ap[0]), [Q, CPB]])
            nc.vector.memset(z_ap, 0.0)
            ones1 = sb.tile([128, 1], F32)
            nc.vector.memset(ones1, 1.0)

            # position-block tiles
            C32 = sb.tile([128, NPOS], F32)
            B32 = sb.tile([128, NPOS], F32)
            x32 = sb.tile([128, 2, NPOS], F32)
            zs = sb.tile([128, 2, NPOS], F32)
            dt32 = sb.tile([HPC, NPOS], F32)
            dAA32 = sb.tile([HPC, NPOS], F32)
            f4 = sb.tile([HPC, NPOS], F32)
            dA_bc = sb.tile([128, 2, NPOS], F32)
            f_bc = sb.tile([128, 2, NPOS], F32)

            # DRAM staging
            w_dd = dsc.tile([HPC, L], BF16)
            lam_d = dsc.tile([HPC, NCHUNK], F32)
            bc_d = dsc.tile([1, NPOS], F32)
            small_d = dsc.tile([2, HPC, NPOS], F32)

            # ---------------- helpers ----------------
            def emit_scan_pair(P):
                """Scan matmuls + incremental Sg updates for pair P."""
                for cc in range(2 * CPB):
                    c = 2 * CPB * P + cc
                    pc2 = psA.tile([128, HPC * HEADDIM], F32, tag="psc")
                    for k2 in range(2):
                        T = 2 * cc + k2
                        nc.tensor.matmul(pc2, BT_r[P % 2][:, T, :],
                                         XT_r[P % 2][:, T, :],
                                         start=(k2 == 0), stop=(k2 == 1))
                    for h in range(HPC):
                        dst = bass.AP(tensor=Sg.tensor,
                                      offset=Sg.offset + h * HPN + c,
                                      ap=[list(Sg.ap[0]), [NCHUNK, HEADDIM]])
                        src = pc2[:, h * HEADDIM:(h + 1) * HEADDIM]
                        if c == 0:
                            nc.vector.tensor_copy(out=dst, in_=src)
                        else:
                            prev = bass.AP(tensor=Sg.tensor,
                                           offset=Sg.offset + h * HPN + c - 1,
                                           ap=[list(Sg.ap[0]), [NCHUNK, HEADDIM]])
                            nc.vector.scalar_tensor_tensor(
                                out=dst, in0=prev, scalar=lam_bc[:, h, c:c + 1],
                                in1=src, op0=OP.mult, op1=OP.add)

            def emit_pos_block():
                """All position-wise quantities from window/pos matmuls."""
                # window matmuls: C, B, x0, x1 (pre-conv values at 32x4 tokens)
                pre = []
                specs = [(w_c_sb, None), (w_in_sb, (256, 128)),
                         (w_in_sb, (0, 128)), (w_in_sb, (128, 128))]
                for w_sb, sl in specs:
                    pw = psA.tile([128, NPOS * D_CONV], F32, tag="pcv", bufs=2)
                    for k in range(KT):
                        lhs = w_sb[:, k, :] if sl is None else \
                            w_sb[:, k, sl[0]:sl[0] + sl[1]]
                        nc.tensor.matmul(pw, lhs, xtw_sb[:, k, :],
                                         start=(k == 0), stop=(k == KT - 1))
                    # 4-tap combine on vector -> f32 pre-act [128, NPOS]
                    tp = sb.tile([128, NPOS], F32, tag="tapc", bufs=4)
                    cw = cwc_sb if sl is None else cw3_sb[:, {256: 2, 0: 0, 128: 1}[sl[0]], :]
                    for j in range(D_CONV):
                        src = bass.AP(tensor=pw.tensor, offset=pw.offset + j,
                                      ap=[list(pw.ap[0]), [D_CONV, NPOS]])
                        if j == 0:
                            nc.vector.tensor_scalar_mul(tp, src, cw[:, 0:1])
                        else:
                            nc.vector.scalar_tensor_tensor(
                                out=tp, in0=src, scalar=cw[:, j:j + 1], in1=tp,
                                op0=OP.mult, op1=OP.add)
                    pre.append(tp)
                # z at positions
                pz = psA.tile([128, 2, NPOS], F32, tag="pin0")
                for t in range(2):
                    for k in range(KT):
                        nc.tensor.matmul(pz[:, t, :],
                                         w_z_sb[:, k, t * 128:(t + 1) * 128],
                                         xtp_sb[:, k, :],
                                         start=(k == 0), stop=(k == KT - 1))
                # dt at positions
                pdt32 = psA.tile([HPC, NPOS], F32, tag="pdt", bufs=2)
                for k in range(KT):
                    nc.tensor.matmul(pdt32, w_dt_sb[:, k, :], xtp_sb[:, k, :],
                                     start=(k == 0), stop=(k == KT - 1))
                # silu batch (one table load)
                nc.scalar.activation(out=C32, in_=pre[0], func=AF.Silu,
                                     bias=cbc_sb[:, 0:1], scale=1.0)
                nc.scalar.activation(out=B32, in_=pre[1], func=AF.Silu,
                                     bias=cb_sb[:, 2:3], scale=1.0)
                nc.scalar.activation(out=x32[:, 0, :], in_=pre[2], func=AF.Silu,
                                     bias=cb_sb[:, 0:1], scale=1.0)
                nc.scalar.activation(out=x32[:, 1, :], in_=pre[3], func=AF.Silu,
                                     bias=cb_sb[:, 1:2], scale=1.0)
                nc.scalar.activation(out=zs, in_=pz, func=AF.Silu)
                # dt / dA at positions
                # softplus(v) = ln(1 + exp(v)); exp+ln share one act table
                nc.scalar.activation(out=dt32, in_=pdt32, func=AF.Exp,
                                     bias=dtb_sb[:, 0:1], scale=1.0)
                nc.vector.tensor_scalar_add(dt32, dt32, 1.0)
                nc.scalar.activation(out=dt32, in_=dt32, func=AF.Ln)
                nc.vector.tensor_scalar_mul(dAA32, dt32, A_sb[:, 0:1])
                nc.scalar.activation(out=dAA32, in_=dAA32, func=AF.Exp)
                # BC row -> f4 = dt32*BC + D
                bc_t = sb.tile([128, NPOS], F32, tag="tapc", bufs=4)
                nc.vector.tensor_mul(bc_t, B32, C32)
                pbc = psA.tile([1, NPOS], F32, tag="psc")
                nc.tensor.matmul(pbc, ones1, bc_t, start=True, stop=True)
                bc_row = sb.tile([1, NPOS], F32)
                nc.vector.tensor_copy(out=bc_row, in_=pbc)
                nc.gpsimd.dma_start(out=bc_d, in_=bc_row)
                bc4 = sb.tile([HPC, NPOS], F32)
                nc.gpsimd.dma_start(
                    out=bc4, in_=bass.AP(tensor=bc_d.tensor, offset=bc_d.offset,
                                         ap=[[0, HPC], [1, NPOS]]))
                nc.vector.tensor_mul(f4, dt32, bc4)
                nc.vector.tensor_scalar(f4, f4, D_sb[:, 0:1], None, OP.add)
                # broadcasts of dAA32/f4 -> [128, 2, NPOS]
                nc.gpsimd.dma_start(out=small_d[0], in_=dAA32)
                nc.gpsimd.dma_start(out=small_d[1], in_=f4)
                for col, t_sb in ((0, dA_bc), (1, f_bc)):
                    for t in range(2):
                        for hh in range(2):
                            src = bass.AP(
                                tensor=small_d.tensor,
                                offset=small_d.offset + col * HPC * NPOS
                                + (2 * t + hh) * NPOS,
                                ap=[[0, 64], [1, NPOS]])
                            nc.gpsimd.dma_start(
                                out=t_sb[64 * hh:64 * (hh + 1), t, :], in_=src)

            # ---------------- main loop ----------------
            prev_xbc = None
            for b in range(NB):
                P = b // 2
                if b + 1 < NB:
                    nc.sync.dma_start(
                        out=xt_slots[(b + 1) % 2],
                        in_=xT.rearrange("(k p) t -> p k t", p=128)
                        [:, :, (b + 1) * BSZ:(b + 2) * BSZ])
                xt_b = xt_slots[b % 2]
                # dt matmuls for both groups first
                pdts = []
                for gg in range(2):
                    pdt = psA.tile([HPC, GSZ], F32, tag="pdt", bufs=2)
                    for k in range(KT):
                        nc.tensor.matmul(pdt, w_dt_sb[:, k, :],
                                         xt_b[:, k, gg * GSZ:(gg + 1) * GSZ],
                                         start=(k == 0), stop=(k == KT - 1))
                    pdts.append(pdt)
                # dt chain (scalar/vector; PE proceeds to in_proj below)
                dt2 = dt2_slots[b % 2]
                for gg in range(2):
                    nc.scalar.activation(out=dt2[:, gg * GSZ:(gg + 1) * GSZ],
                                         in_=pdts[gg], func=AF.Exp,
                                         bias=dtb_sb[:, 0:1], scale=1.0)
                nc.vector.tensor_scalar_add(dt2, dt2, 1.0)
                nc.scalar.activation(out=dt2, in_=dt2, func=AF.Ln)
                nc.vector.tensor_scalar_mul(a2_s, dt2, A_sb[:, 0:1])
                nc.vector.tensor_tensor_scan(out=s2_s, data0=ones4, data1=a2_s,
                                             initial=0.0, op0=OP.mult, op1=OP.add)
                cpos = b * CPB
                src = bass.AP(tensor=s2_s.tensor, offset=s2_s.offset + Q - 1,
                              ap=[list(s2_s.ap[0]), [Q, CPB]])
                nc.scalar.activation(out=lamA[:, cpos:cpos + CPB], in_=src,
                                     func=AF.Exp)
                for cc in range(CPB):
                    stot = bass.AP(tensor=s2_s.tensor,
                                   offset=s2_s.offset + cc * Q + Q - 1,
                                   ap=[list(s2_s.ap[0]), [1, 1]])
                    nc.vector.tensor_scalar(s2_s[:, cc * Q:(cc + 1) * Q],
                                            s2_s[:, cc * Q:(cc + 1) * Q],
                                            stot, None, OP.subtract)
                nc.scalar.activation(out=s2_s, in_=s2_s, func=AF.Exp, scale=-1.0)
                nc.vector.tensor_mul(s2_s, s2_s, dt2)
                w2b = w2b_slots[b % 2]
                nc.vector.tensor_copy(out=w2b, in_=s2_s)
                # lam broadcast staging (consumed a pair later)
                nc.gpsimd.dma_start(out=lam_d[:, cpos:cpos + CPB],
                                    in_=lamA[:, cpos:cpos + CPB])
                lam_src = bass.AP(tensor=lam_d.tensor, offset=lam_d.offset + cpos,
                                  ap=[[0, 128], [NCHUNK, HPC], [1, CPB]])
                nc.gpsimd.dma_start(out=lam_bc[:, :, cpos:cpos + CPB], in_=lam_src)
                # w broadcast: DRAM roundtrip -> [128, 2, BSZ]
                bsl = slice(b * BSZ, (b + 1) * BSZ)
                nc.gpsimd.dma_start(out=w_dd[:, bsl], in_=w2b)
                wbc2 = ring.tile([128, 2, BSZ], BF16, tag="wbc", bufs=2)
                for t in range(2):
                    src = bass.AP(tensor=w_dd.tensor,
                                  offset=w_dd.offset + (2 * t) * L + b * BSZ,
                                  ap=[[L, 2], [0, 64], [1, BSZ]])
                    nc.gpsimd.dma_start(out=wbc2[:, t, :], in_=src)
                # in_proj + conv per group
                for gg in range(2):
                    # scan matmuls for the previous pair between the two
                    # groups (its transposes then have ~1.5 batches of slack)
                    if gg == 1 and b % 2 == 0 and b >= 2:
                        emit_scan_pair(P - 1)
                    g = 2 * b + gg
                    xt_g = xt_b[:, :, gg * GSZ:(gg + 1) * GSZ]
                    ps = []
                    for mt, (c0, cwd) in enumerate(MT_SPEC):
                        p = psA.tile([cwd, GSZ], F32, tag=f"pin{mt}")
                        for k in range(KT):
                            nc.tensor.matmul(p, w_in_sb[:, k, c0:c0 + cwd],
                                             xt_g[:, k, :],
                                             start=(k == 0), stop=(k == KT - 1))
                        ps.append(p)
                    xbc_g = ring.tile([128, 3, GSZ + 3], BF16, tag="xbc", bufs=2)
                    if prev_xbc is None:
                        nc.vector.memset(xbc_g[:, :, 0:3], 0.0)
                    else:
                        nc.vector.tensor_copy(out=xbc_g[:, :, 0:3],
                                              in_=prev_xbc[:, :, GSZ:GSZ + 3])
                    for cht in range(3):
                        nc.vector.tensor_copy(out=xbc_g[:, cht, 3:], in_=ps[cht])
                    prev_xbc = xbc_g
                    goff = (2 * (b % 2) + gg) * GSZ  # offset within pair buffer
                    for cht in range(3):
                        pc = psA.tile([128, GSZ], F32, tag="pcv", bufs=2)
                        for j in range(D_CONV):
                            nc.tensor.matmul(pc, diag_sb[:, cht, j, :],
                                             xbc_g[:, cht, j:j + GSZ],
                                             start=(j == 0), stop=(j == D_CONV - 1))
                        nc.scalar.activation(
                            out=xbcc[P % 2][:, cht, goff:goff + GSZ], in_=pc,
                            func=AF.Silu, bias=cb_sb[:, cht:cht + 1], scale=1.0)
                # scale + transpose for this batch
                boff = (b % 2) * BSZ
                xs2 = ring.tile([128, 2, BSZ], BF16, tag="xs", bufs=2)
                for t in range(2):
                    nc.vector.tensor_mul(xs2[:, t, :],
                                         xbcc[P % 2][:, t, boff:boff + BSZ],
                                         wbc2[:, t, :])
                    eng = nc.sync if t == 0 else nc.scalar
                    eng.dma_start_transpose(
                        out=XT_r[P % 2][:, 8 * (b % 2):8 * (b % 2) + 8,
                                        t * 128:(t + 1) * 128],
                        in_=xs2[:, t, :])
                if b % 2 == 1:
                    nc.scalar.dma_start_transpose(out=BT_r[P % 2],
                                                  in_=xbcc[P % 2][:, 2, :])
                if b == 0:
                    emit_pos_block()

            # ---------------- tail ----------------
            emit_scan_pair(NPAIR - 1)
            # per-position projections py[(hh,p), t, pos] = C32 . Sg[:,(h,p,pos-1)]
            py = psA.tile([128, 2, NPOS], F32, tag="psc")
            nc.vector.memset(py[:, :, 0:1], 0.0)
            first = True
            for pos in range(1, NPOS):
                for t in range(2):
                    lhs = bass.AP(
                        tensor=Sg.tensor,
                        offset=Sg.offset + (2 * t) * HPN + (pos - 1),
                        ap=[list(Sg.ap[0]), [HPN, 2], [NCHUNK, 64]])
                    nc.tensor.matmul(py[:, t, pos:pos + 1], lhs,
                                     C32[:, pos:pos + 1],
                                     start=first, stop=(pos == NPOS - 1 and t == 1),
                                     skip_group_check=True)
                    first = False
            y32 = sb.tile([128, 2, NPOS], F32)
            nc.vector.tensor_mul(y32, py, dA_bc)
            tloc = sb.tile([128, 2, NPOS], F32)
            nc.vector.tensor_mul(tloc, x32, f_bc)
            nc.vector.tensor_add(y32, y32, tloc)
            nc.vector.tensor_mul(y32, y32, zs)
            nc.sync.dma_start(out=y32g_out, in_=y32)

    nc.compile()
    _L1 = nc
    return nc


# ----------------------------------------------------------------------------
# Launch 2 program: gated RMSNorm + out_proj + MLP on the 32 rows
# ----------------------------------------------------------------------------
_L2 = None


def build_l2():
    global _L2
    if _L2 is not None:
        return _L2
    nc = bacc.Bacc("TRN2", target_bir_lowering=False, debug=False,
                   num_devices=NCORES)

    y32g = nc.dram_tensor("y32g_full", (128, 16, NPOS), F32, kind="ExternalInput").ap()
    norm_w = nc.dram_tensor("norm_w", (128, 16), F32, kind="ExternalInput").ap()
    w_out = nc.dram_tensor("w_outT", (16, 128, D_MODEL), BF16, kind="ExternalInput").ap()
    w1 = nc.dram_tensor("w1T", (KT, 128, HIDDEN), BF16, kind="ExternalInput").ap()
    b1 = nc.dram_tensor("b1", (128, HIDDEN // 128), F32, kind="ExternalInput").ap()
    w2 = nc.dram_tensor("w2T", (32, 128, 512), BF16, kind="ExternalInput").ap()
    b2 = nc.dram_tensor("b2", (128, 4), F32, kind="ExternalInput").ap()
    out32 = nc.dram_tensor("out32", (128, 4, NPOS), F32, kind="ExternalOutput").ap()

    with tile.TileContext(nc) as tc:
        import contextlib
        with contextlib.ExitStack() as ctx:
            sb = ctx.enter_context(tc.tile_pool(name="sb", bufs=1))
            psp = ctx.enter_context(tc.tile_pool(name="ps", bufs=1, space="PSUM"))
            dsc = ctx.enter_context(tc.tile_pool(name="dsc", bufs=1, space="DRAM"))

            y_sb = sb.tile([128, 16, NPOS], F32)
            nc.sync.dma_start(out=y_sb, in_=y32g)
            nw_sb = sb.tile([128, 16], F32)
            nc.sync.dma_start(out=nw_sb, in_=norm_w)
            wo_sb = sb.tile([128, 16, D_MODEL], BF16)
            nc.sync.dma_start(out=wo_sb, in_=w_out.rearrange("k p c -> p k c"))
            w1_sb = sb.tile([128, KT, HIDDEN], BF16)
            nc.sync.dma_start(out=w1_sb, in_=w1.rearrange("k p c -> p k c"))
            b1_sb = sb.tile([128, HIDDEN // 128], F32)
            nc.sync.dma_start(out=b1_sb, in_=b1)
            w2_sb = sb.tile([128, 32, 512], BF16)
            nc.sync.dma_start(out=w2_sb, in_=w2.rearrange("k p c -> p k c"))
            b2_sb = sb.tile([128, 4], F32)
            nc.sync.dma_start(out=b2_sb, in_=b2)

            # sum of squares over channels (partition x 16 ktiles)
            sq = sb.tile([128, 16, NPOS], F32)
            nc.vector.tensor_mul(sq, y_sb, y_sb)
            ones1 = sb.tile([128, 1], F32)
            nc.vector.memset(ones1, 1.0)
            pss = psp.tile([1, NPOS], F32, tag="pss")
            for k in range(16):
                nc.tensor.matmul(pss, ones1, sq[:, k, :],
                                 start=(k == 0), stop=(k == 15))
            # r = 1/sqrt(mean + eps)
            eps_t = sb.tile([1, 1], F32)
            nc.vector.memset(eps_t, 1e-5)
            rs = sb.tile([1, NPOS], F32)
            nc.scalar.activation(out=rs, in_=pss, func=AF.Sqrt,
                                 bias=eps_t[:, 0:1], scale=1.0 / D_INNER)
            nc.vector.reciprocal(rs, rs)
            r_d = dsc.tile([1, NPOS], F32)
            nc.sync.dma_start(out=r_d, in_=rs)
            r_bc = sb.tile([128, NPOS], F32)
            nc.sync.dma_start(out=r_bc,
                              in_=bass.AP(tensor=r_d.tensor, offset=r_d.offset,
                                          ap=[[0, 128], [1, NPOS]]))
            yn = sb.tile([128, 16, NPOS], BF16)
            for k in range(16):
                nc.vector.scalar_tensor_tensor(out=yn[:, k, :], in0=y_sb[:, k, :],
                                               scalar=nw_sb[:, k:k + 1], in1=r_bc,
                                               op0=OP.mult, op1=OP.mult)
            # h = w_outT.T @ yn   [1024, 32]
            h_sb = sb.tile([128, 8, NPOS], BF16)
            for mt in range(8):
                ph = psp.tile([128, NPOS], F32, tag="ph", bufs=2)
                for k in range(16):
                    nc.tensor.matmul(ph, wo_sb[:, k, mt * 128:(mt + 1) * 128],
                                     yn[:, k, :], start=(k == 0), stop=(k == 15))
                nc.vector.tensor_copy(out=h_sb[:, mt, :], in_=ph)
            # g = gelu(w1T.T @ h + b1)  [4096, 32]
            g_sb = sb.tile([128, 32, NPOS], BF16)
            for mt in range(32):
                pg = psp.tile([128, NPOS], F32, tag="pg", bufs=2)
                for k in range(KT):
                    nc.tensor.matmul(pg, w1_sb[:, k, mt * 128:(mt + 1) * 128],
                                     h_sb[:, k, :], start=(k == 0), stop=(k == KT - 1))
                nc.scalar.activation(out=g_sb[:, mt, :], in_=pg, func=AF.Gelu,
                                     bias=b1_sb[:, mt:mt + 1], scale=1.0)
            # out = w2T.T @ g + b2   [512, 32] per core
            for mt in range(4):
                po = psp.tile([128, NPOS], F32, tag="po", bufs=2)
                for k in range(32):
                    nc.tensor.matmul(po, w2_sb[:, k, mt * 128:(mt + 1) * 128],
                                     g_sb[:, k, :], start=(k == 0), stop=(k == 31))
                ot = sb.tile([128, NPOS], F32, tag="ot", bufs=2)
                nc.vector.tensor_scalar(ot, po, b2_sb[:, mt:mt + 1], None, OP.add)
                nc.sync.dma_start(out=out32[:, mt, :], in_=ot)

    nc.compile()
    _L2 = nc
    return nc


# ----------------------------------------------------------------------------
# Host-side prep + glue
# ----------------------------------------------------------------------------

def _prep_l1_maps(inputs):
    x = _f32(inputs["x"]).reshape(L, D_MODEL)
    xT = np.ascontiguousarray(x.T)                       # [1024, 8192]
    xT_b = _bf(xT)
    pos = np.arange(NPOS) * POS_STRIDE
    xTpos = _bf(xT[:, pos])
    # window tokens (pos, d): t*-3+d, zero-padded below 0
    win_idx = (pos[:, None] + np.arange(D_CONV)[None, :] - (D_CONV - 1)).reshape(-1)
    xTwin = np.zeros((D_MODEL, NPOS * D_CONV), np.float32)
    valid = win_idx >= 0
    xTwin[:, valid] = xT[:, win_idx[valid]]
    xTwin = _bf(xTwin)

    w_all = _f32(inputs["in_proj_w"])                    # [4384, 1024]
    conv_w = _f32(inputs["conv_w"])                      # [2304, 4]
    conv_b = _f32(inputs["conv_b"])                      # [2304]
    dt_bias = _f32(inputs["dt_bias"])                    # [32]
    A = -np.exp(_f32(inputs["A_log"]))                   # [32]
    Dp = _f32(inputs["D"])                               # [32]

    w_cT = _bf(w_all[D_INNER + D_INNER + D_STATE:
                     D_INNER + D_INNER + 2 * D_STATE].T.reshape(KT, 128, 128))
    cw_c = _f32(conv_w[D_INNER + D_STATE:])              # [128, 4] C channels
    conv_b_c = _f32(conv_b[D_INNER + D_STATE:]).reshape(128, 1)

    maps = []
    for k in range(NCORES):
        xs = 256 * k
        cols = np.concatenate([
            np.arange(D_INNER + xs, D_INNER + xs + 256),          # x slice
            np.arange(2 * D_INNER, 2 * D_INNER + D_STATE),        # B
        ])
        w_in = _bf(w_all[cols].T.reshape(KT, 128, NCOL))
        dt_cols = np.arange(D_IN_PROJ - NHEADS + HPC * k,
                            D_IN_PROJ - NHEADS + HPC * k + HPC)
        w_dt = _bf(w_all[dt_cols].T.reshape(KT, 128, HPC))
        w_z = _bf(w_all[xs:xs + 256].T.reshape(KT, 128, 256))
        # conv channels for this core: x slice (256) + B (128)
        ch_x = np.arange(xs, xs + 256)
        ch_B = np.arange(D_INNER, D_INNER + D_STATE)
        dw = np.zeros((3, D_CONV, 128, 128), np.float32)
        cb = np.zeros((128, 3), np.float32)
        cw3 = np.zeros((128, 3, D_CONV), np.float32)
        for cht, chs in enumerate([ch_x[:128], ch_x[128:], ch_B]):
            for j in range(D_CONV):
                dw[cht, j] = np.diag(conv_w[chs, j])
            cb[:, cht] = conv_b[chs]
            cw3[:, cht, :] = conv_w[chs, :]
        heads = np.arange(HPC * k, HPC * k + HPC)
        maps.append({
            "xT": xT_b, "xTpos": xTpos, "xTwin": xTwin,
            "w_in": w_in, "w_dt": w_dt, "w_c": w_cT, "w_z": w_z,
            "diag_w": _bf(dw), "cw_c": cw_c, "cw3": cw3, "conv_b": cb,
            "conv_b_c": conv_b_c,
            "dtb4": dt_bias[heads].reshape(HPC, 1).astype(np.float32),
            "A4": A[heads].reshape(HPC, 1).astype(np.float32),
            "D4": Dp[heads].reshape(HPC, 1).astype(np.float32),
        })
    return maps


def _prep_l2_maps(inputs, y32g_full):
    # ch = kt*128 + p -> norm_w_sb[p, kt] = norm_w[kt*128+p]
    nw = _f32(inputs["norm_w"]).reshape(16, 128).transpose(1, 0).copy()
    w_out = _f32(inputs["mamba_out_w"])                  # [1024, 2048]
    w_outT = _bf(w_out.T.reshape(16, 128, D_MODEL))
    w1 = _f32(inputs["mlp_w1"])                          # [4096, 1024]
    w1T = _bf(w1.T.reshape(KT, 128, HIDDEN))
    b1 = _f32(inputs["mlp_b1"]).reshape(32, 128).transpose(1, 0).copy()
    w2 = _f32(inputs["mlp_w2"])                          # [4096, 4096]
    maps = []
    for k in range(NCORES):
        cols = slice(512 * k, 512 * k + 512)
        w2T = _bf(w2[cols].T.reshape(32, 128, 512))
        b2 = _f32(inputs["mlp_b2"])[cols].reshape(4, 128).transpose(1, 0).copy()
        maps.append({
            "y32g_full": y32g_full, "norm_w": nw, "w_outT": w_outT,
            "w1T": w1T, "b1": b1, "w2T": w2T, "b2": b2,
        })
    return maps


LAST_RESULTS = []


def kernel(**inputs) -> np.ndarray:
    trace = os.environ.get("KERNEL_TRACE", "0") == "1"
    LAST_RESULTS.clear()
    nc1 = build_l1()
    maps1 = _prep_l1_maps(inputs)
    res1 = bass_utils.run_bass_kernel_spmd(nc1, maps1, core_ids=list(range(NCORES)),
                                           trace=trace)
    LAST_RESULTS.append(res1)
    # assemble y32g_full [128, 16, 32]: ch = 256*k + t*128 + p -> kt = 2k+t
    y32g_full = np.zeros((128, 16, NPOS), np.float32)
    for k in range(NCORES):
        y = res1.results[k]["y32g"]                      # [128, 2, 32]
        y32g_full[:, 2 * k:2 * k + 2, :] = y
    nc2 = build_l2()
    maps2 = _prep_l2_maps(inputs, y32g_full)
    res2 = bass_utils.run_bass_kernel_spmd(nc2, maps2, core_ids=list(range(NCORES)),
                                           trace=trace)
    LAST_RESULTS.append(res2)
    out = np.zeros((NPOS, HIDDEN), np.float32)
    for k in range(NCORES):
        o = res2.results[k]["out32"]                     # [128, 4, 32]
        # out[pos, 512k + mt*128 + p] = o[p, mt, pos]
        out[:, 512 * k:512 * (k + 1)] = o.transpose(2, 1, 0).reshape(NPOS, 512)
    return out.astype(np.float32)
